# revision 1
# baseline (speedup 1.0000x reference)
"""Trainium2 Bass kernel for nn_DataEmbedding_v2 (circular conv1d + sinusoidal PE
+ causal threshold-scan "tc" embedding).

Contract: kernel(**inputs) takes FULL inputs (x:(16,2048,64) f32, conv_w:(512,64,3),
tc_w:(512,1), tc_b:(512,)) and returns the FULL (16,2048,512) f32 output.
Internally: batch-sharded data-parallel across 8 NeuronCores (2 batches/core),
params replicated.

Numerics: the tc threshold compare runs in exact fp32 (bit-matching the
reference's rounding); conv weights/activations and pe run in fp16 on the PE
(absolute error ~1e-3 vs output absmax ~1.2e3); tau*tc_w uses an fp16 hi+lo
split of tc_w so the (up to 2048x) tau amplification stays at fp32-level error.
"""

import math
import os
import sys

sys.path.insert(0, "/opt/trn_rl_repo")

import numpy as np

B, S, C, D = 16, 2048, 64, 512
NCORES = 8
BLOC = B // NCORES  # batches per core
P = 128
NT = S // P  # 16 tiles of 128 tokens
ETA = 0.3
EPS = 1e-08


def _emit(tc, aps):
    """Emit the per-core Tile kernel. aps: dict of DRAM APs."""
    from contextlib import ExitStack

    from concourse import mybir

    f32 = mybir.dt.float32
    f16 = mybir.dt.float16
    Alu = mybir.AluOpType
    Ax = mybir.AxisListType

    nc = tc.nc
    xin, pe16, wt01, wt2 = aps["xin"], aps["pe16"], aps["wt01"], aps["wt2"]
    tcwhl, identh, causal = aps["tcwhl"], aps["identh"], aps["causal"]
    splus1, t1c, out = aps["splus1"], aps["t1c"], aps["out"]
    th_dram, tau_dram = aps["th_dram"], aps["tau_dram"]

    with ExitStack() as ctx:
        singles = ctx.enter_context(tc.tile_pool(name="singles", bufs=1))
        xpool = ctx.enter_context(tc.tile_pool(name="xpool", bufs=2))
        small = ctx.enter_context(tc.tile_pool(name="small", bufs=2))
        rowpool = ctx.enter_context(tc.tile_pool(name="rowpool", bufs=2))
        xtp = ctx.enter_context(tc.tile_pool(name="xtp", bufs=2))
        accp = ctx.enter_context(tc.tile_pool(name="accp", bufs=2))
        genp = ctx.enter_context(tc.tile_pool(name="genp", bufs=4))
        outp = ctx.enter_context(tc.tile_pool(name="outp", bufs=6))
        psA = ctx.enter_context(tc.tile_pool(name="psA", bufs=3, space="PSUM"))
        psT = ctx.enter_context(tc.tile_pool(name="psT", bufs=3, space="PSUM"))
        psX = ctx.enter_context(tc.tile_pool(name="psX", bufs=2, space="PSUM"))

        # ---- x loads first (keep the sync DMA ring clear for the
        # norm/thresh critical path; consts follow) ----
        xins = {}
        for b in range(BLOC):
            xin_sb = xpool.tile([P, NT, C], f32, tag="xin", name=f"xin_sb{b}")
            nc.sync.dma_start(xin_sb, xin[b].rearrange("(j p) c -> p j c", p=P))
            xins[b] = xin_sb

        # ---- constants to SBUF ----
        identh_sb = singles.tile([P, P], f16)
        nc.sync.dma_start(identh_sb, identh)
        causal_sb = singles.tile([P, P], f16)
        nc.sync.dma_start(causal_sb, causal)
        splus1_sb = singles.tile([P, NT], f32)
        nc.sync.dma_start(splus1_sb, splus1)
        t1_sb = singles.tile([P, NT], f32)
        nc.sync.dma_start(t1_sb, t1c)
        wt01_sb = singles.tile([P, D], f16)
        nc.sync.dma_start(wt01_sb, wt01)
        wt2_sb = singles.tile([P, D], f16)
        nc.sync.dma_start(wt2_sb[C : 2 * C, :], wt2)
        # tc_w hi/lo rows on partitions 32,33: the K=2 tau matmul row-packs
        # into array rows 32-33, concurrent with the K=64 tap2 matmul (rows
        # 64-127) inside the same accumulation group
        tcwhl_sb = singles.tile([34, D], f16)
        nc.sync.dma_start(tcwhl_sb[32:34, :], tcwhl[0])
        pe16_sb = singles.tile([P, NT, D], f16)
        nc.sync.dma_start(pe16_sb, pe16.rearrange("(i p) d -> p i d", p=P))

        # ---- PE priming ----
        # HW limit: a PE matmul/transpose carries at most ONE sync wait.
        # Dedicated ops absorb each const-DMA dependency; priming PSUM outputs
        # are read by ScalarE so later bank reuse waits on ACT.
        prime_h = psT.tile([P, P], f16, tag="pst")
        nc.tensor.transpose(prime_h, identh_sb, identh_sb)
        prime_w2 = psT.tile([P, D], f32, tag="pst")
        nc.tensor.matmul(
            prime_w2[C : 2 * C, :],
            lhsT=wt2_sb[C : 2 * C, 0:C],
            rhs=wt2_sb[C : 2 * C, :],
            start=True,
            stop=True,
        )
        prime_tc = psT.tile([P, D], f32, tag="pst")
        nc.tensor.matmul(
            prime_tc,
            lhsT=tcwhl_sb[32:34, 0:P],
            rhs=tcwhl_sb[32:34, :],
            start=True,
            stop=True,
            tile_position=(32, 0),
        )
        prime_w01 = psA.tile([P, D], f32, tag="psa")
        nc.tensor.matmul(
            prime_w01, lhsT=wt01_sb[:, 0:P], rhs=wt01_sb, start=True, stop=True
        )
        prime_pe = psA.tile([P, D], f32, tag="psa")
        nc.tensor.matmul(
            prime_pe, lhsT=identh_sb, rhs=pe16_sb[:, 0, :], start=True, stop=True
        )
        dumps = singles.tile([P, 5], f32)
        nc.scalar.copy(dumps[:, 0:1], prime_h[:, 0:1])
        nc.scalar.copy(dumps[C : 2 * C, 1:2], prime_w2[C : 2 * C, 0:1])
        nc.scalar.copy(dumps[:, 2:3], prime_tc[:, 0:1])
        nc.scalar.copy(dumps[:, 3:4], prime_w01[:, 0:1])
        nc.scalar.copy(dumps[:, 4:5], prime_pe[:, 0:1])

        # ---- Phase A (both batches): load x, norms/thresh, xTA build ----
        # Emitting both batches' input chains before any conv keeps the sync
        # DMA ring free of output traffic, so batch 1's tc work can overlap
        # batch 0's conv phase.
        st = {}
        for b in range(BLOC):
            xin_sb = xins[b]
            # norms (L1 over channels), two-level sum for accuracy
            r8 = small.tile([P, NT, 8], f32, tag="r8", name=f"r8_{b}")
            nc.vector.tensor_reduce(
                r8,
                xin_sb.rearrange("p j (a b) -> p j a b", b=8),
                axis=Ax.X,
                op=Alu.add,
                apply_absolute_value=True,
            )
            normc = small.tile([P, NT], f32, tag="normc", name=f"normc{b}")
            nc.vector.tensor_reduce(normc, r8, axis=Ax.X, op=Alu.add)
            # thresh = (norms + EPS) * 0.7  (exact fp32 rounding order of ref)
            thc = small.tile([P, NT], f32, tag="thc", name=f"thc{b}")
            nc.vector.tensor_scalar(
                thc, normc, float(EPS), float(1.0 - ETA), op0=Alu.add, op1=Alu.mult
            )
            # roundtrip through DRAM to get thresh broadcast over partitions
            nc.sync.dma_start(th_dram[b].rearrange("(j p) -> p j", p=P), thc)
            throw = rowpool.tile([P, S], f32, tag="throw", name=f"throw{b}")
            nc.gpsimd.dma_start(throw, th_dram[b].partition_broadcast(P))

            # xTA: fp16 x, channel-major, two stacked tap views
            # rows 0:64 = x[u-1, c] (tap0 at col u=t); rows 64:128 = x[u, c]
            xin16 = xpool.tile([P, NT, C], f16, tag="xin16", name=f"xin16_{b}")
            nc.vector.tensor_copy(xin16, xin_sb)
            nc.tensor.ldweights(xin16[:, 0, :])  # absorb DVE wait for PE
            xTA = xtp.tile([P, S + 2], f16, tag="xta", name=f"xTA{b}")
            for j in range(NT):
                pt = psX.tile([C, P], f16, tag="psx", name=f"pt{b}_{j}")
                nc.tensor.transpose(pt, xin16[:, j, :], identh_sb)
                nc.scalar.copy(xTA[0:C, 1 + j * P : 1 + (j + 1) * P], pt)
                nc.scalar.copy(xTA[C : 2 * C, j * P : (j + 1) * P], pt)
            nc.scalar.copy(xTA[0:C, 0:1], xTA[0:C, S : S + 1])
            nc.scalar.copy(xTA[0:C, S + 1 : S + 2], xTA[0:C, 1:2])
            nc.scalar.copy(xTA[C : 2 * C, S : S + 1], xTA[C : 2 * C, 0:1])
            st[b] = (normc, throw, xTA)

        # ---- Phase B (per batch): tc plane, tau, conv, output ----
        for b in range(BLOC):
            normc, throw, xTA = st[b]
            # tc plane: ACC[s_loc, t] = max_j (thresh[t] > norms[s])*(s+1)
            ACC = accp.tile([P, S], f16, tag="acc", name=f"ACC{b}")
            for j in range(NT):
                c0 = j * P
                if j == 0:
                    gj = ACC
                else:
                    gj = genp.tile([P, S - P], f16, tag="gj", name=f"gj{b}_{j}")
                gslice = gj[:, 0 : S - c0] if j > 0 else gj
                nc.vector.tensor_scalar(
                    gslice,
                    throw[:, c0:S],
                    normc[:, j : j + 1],
                    splus1_sb[:, j : j + 1],
                    op0=Alu.is_gt,
                    op1=Alu.mult,
                )
                # mask s >= t inside the diagonal 128 columns
                nc.vector.tensor_tensor(
                    gj[:, 0:P], gj[:, 0:P], causal_sb, op=Alu.mult
                )
                if j > 0:
                    nc.vector.tensor_tensor(
                        ACC[:, c0:S], gj[:, 0 : S - c0], ACC[:, c0:S], op=Alu.max
                    )

            # ---- cross-partition max via PE transpose + free-dim reduce ----
            nc.tensor.ldweights(ACC[:, 0:P])  # absorb the DVE->PE ACC wait
            rc = small.tile([P, NT], f32, tag="rc", name=f"rc{b}")
            for g in range(4):
                pT = psT.tile([P, 4 * P], f16, tag="pst", name=f"pT{b}_{g}")
                for m in range(4):
                    nc.tensor.transpose(
                        pT[:, m * P : (m + 1) * P],
                        ACC[:, g * 4 * P + m * P : g * 4 * P + (m + 1) * P],
                        identh_sb,
                    )
                nc.vector.tensor_reduce(
                    rc[:, g * 4 : (g + 1) * 4],
                    pT.rearrange("p (m q) -> p m q", q=P),
                    axis=Ax.X,
                    op=Alu.max,
                )

            # ---- tau = (r > 0) ? (t + 1 - r) : 0 ----
            m01 = small.tile([P, NT], f32, tag="m01", name=f"m01_{b}")
            nc.vector.tensor_scalar(m01, rc, 0.0, None, op0=Alu.is_gt)
            td = small.tile([P, NT], f32, tag="td", name=f"td{b}")
            nc.vector.tensor_tensor(td, t1_sb, rc, op=Alu.subtract)
            tauc = small.tile([P, NT], f32, tag="tauc", name=f"tauc{b}")
            nc.vector.tensor_tensor(tauc, td, m01, op=Alu.mult)
            # tau as fp16 rows on partitions 64,65 via DRAM roundtrip
            # (values are integers <= 2048: exact in fp16)
            # contiguous p-major write (addr = p*NT + i); the conv lhsT view
            # below un-permutes with a stride-16 access pattern
            nc.sync.dma_start(tau_dram[b].rearrange("(p j) -> p j", p=P), tauc)
            taurow = small.tile([34, S], f16, tag="taurow", name=f"taurow{b}")
            nc.gpsimd.dma_start(taurow[32:34, :], tau_dram[b].partition_broadcast(2))
            taujp = taurow[32:34, :].rearrange("q (p j) -> q j p", j=NT)
            # absorb taurow-DMA wait before the conv matmuls
            nc.tensor.ldweights(taujp[:, 0, :], tile_position=(32, 0))

            # ---- per tile: pe + conv (2 stacked taps) + tau*(w_hi+w_lo);
            # the K=2 tau matmul row-packs with the K=64 tap2 matmul ----
            for i in range(NT):
                ps = psA.tile([P, D], f32, tag="psa", name=f"ps{b}_{i}")
                nc.tensor.matmul(
                    ps, lhsT=identh_sb, rhs=pe16_sb[:, i, :], start=True, stop=False
                )
                nc.tensor.matmul(
                    ps,
                    lhsT=xTA[:, i * P : (i + 1) * P],
                    rhs=wt01_sb,
                    start=False,
                    stop=False,
                )
                nc.tensor.matmul(
                    ps,
                    lhsT=xTA[C : 2 * C, i * P + 1 : (i + 1) * P + 1],
                    rhs=wt2_sb[C : 2 * C, :],
                    start=False,
                    stop=False,
                )
                nc.tensor.matmul(
                    ps,
                    lhsT=taujp[:, i, :],
                    rhs=tcwhl_sb[32:34, :],
                    start=False,
                    stop=True,
                    tile_position=(32, 0),
                )
                osb = outp.tile([P, D], f32, tag="osb", name=f"osb{b}_{i}")
                nc.scalar.copy(osb, ps)
                nc.scalar.dma_start(out[b, i * P : (i + 1) * P, :], osb)


def build_bass():
    """Build the per-core Bass module (traced once, then bacc-compiled)."""
    import concourse.tile as tile
    from concourse import bacc, mybir

    f32 = mybir.dt.float32
    f16 = mybir.dt.float16

    nc = bacc.Bacc(
        "TRN2",
        target_bir_lowering=False,
        debug=False,
        enable_asserts=False,
        num_devices=NCORES,
    )
    aps = {}
    aps["xin"] = nc.dram_tensor("xin", (BLOC, S, C), f32, kind="ExternalInput").ap()
    aps["pe16"] = nc.dram_tensor("pe16", (S, D), f16, kind="ExternalInput").ap()
    aps["wt01"] = nc.dram_tensor("wt01", (P, D), f16, kind="ExternalInput").ap()
    aps["wt2"] = nc.dram_tensor("wt2", (C, D), f16, kind="ExternalInput").ap()
    aps["tcwhl"] = nc.dram_tensor("tcwhl", (1, 2, D), f16, kind="ExternalInput").ap()
    aps["identh"] = nc.dram_tensor("identh", (P, P), f16, kind="ExternalInput").ap()
    aps["causal"] = nc.dram_tensor("causal", (P, P), f16, kind="ExternalInput").ap()
    aps["splus1"] = nc.dram_tensor("splus1", (P, NT), f32, kind="ExternalInput").ap()
    aps["t1c"] = nc.dram_tensor("t1c", (P, NT), f32, kind="ExternalInput").ap()
    aps["out"] = nc.dram_tensor("out", (BLOC, S, D), f32, kind="ExternalOutput").ap()
    aps["th_dram"] = nc.dram_tensor("th_scratch", (BLOC, S), f32, kind="Internal").ap()
    aps["tau_dram"] = nc.dram_tensor(
        "tau_scratch", (BLOC, S), f32, kind="Internal"
    ).ap()

    with tile.TileContext(nc) as tc:
        _emit(tc, aps)
    nc.compile()
    return nc


def make_consts():
    """Host-side constant tensors (replicated params + index helpers)."""
    # positional embedding, matching the reference formula in fp32
    position = np.arange(S, dtype=np.float32)[:, None]
    div_term = np.exp(
        np.arange(0, D, 2, dtype=np.float32) * np.float32(-math.log(10000.0) / D)
    ).astype(np.float32)
    ang = (position * div_term).astype(np.float32)
    pe = np.zeros((S, D), dtype=np.float32)
    pe[:, 0::2] = np.sin(ang)
    pe[:, 1::2] = np.cos(ang)

    sl = np.arange(P, dtype=np.float32)[:, None]
    jj = np.arange(NT, dtype=np.float32)[None, :]
    consts = {
        "identh": np.eye(P, dtype=np.float16),
        "causal": (np.arange(P)[:, None] < np.arange(P)[None, :]).astype(np.float16),
        "splus1": (jj * P + sl + 1.0).astype(np.float32),
        "t1c": (jj * P + sl + 1.0).astype(np.float32),
    }
    return pe, consts


def make_shared_inputs(conv_w, tc_w, tc_b):
    pe, consts = make_consts()
    pe_b = (pe + np.asarray(tc_b, np.float32)[None, :]).astype(np.float32)
    # conv weights, channel-major per tap: wk[c, d] = conv_w[d, c, k]
    wt = np.transpose(np.asarray(conv_w, np.float32), (2, 1, 0))  # (k, c, d)
    wt01 = np.concatenate([wt[0], wt[1]], axis=0).astype(np.float16)  # (128, D)
    wt2 = wt[2].astype(np.float16)  # (64, D)
    # tc_w split into fp16 hi+lo (tau amplifies errors by up to 2048)
    w = np.asarray(tc_w, np.float32)[:, 0]
    w_hi = w.astype(np.float16)
    w_lo = (w - w_hi.astype(np.float32)).astype(np.float16)
    tcwhl = np.stack([w_hi, w_lo], axis=0)[None]  # (1, 2, D)
    return {
        "pe16": pe_b.astype(np.float16),
        "wt01": np.ascontiguousarray(wt01),
        "wt2": np.ascontiguousarray(wt2),
        "tcwhl": np.ascontiguousarray(tcwhl),
        **{k: np.ascontiguousarray(v) for k, v in consts.items()},
    }


_BUILD_CACHE = {}


def _install_ntff_hook():
    """The agent image's antenv lacks axon_hooks; synthesize it from the
    boot module's ctypes implementation so trace=True works under axon."""
    import sys as _sys
    import types

    if "antenv.axon_hooks" in _sys.modules:
        return
    try:
        from trn_agent_boot.trn_boot import _ntff_profile_via_ctypes

        hook = _ntff_profile_via_ctypes("/opt/axon/libaxon_pjrt.so")
        m = types.ModuleType("antenv.axon_hooks")
        m.get_axon_ntff_profile_hook = lambda: hook
        _sys.modules["antenv.axon_hooks"] = m
    except Exception as e:  # degrade to no-trace
        print("[kernel] ntff hook install failed:", e)


def kernel(x, conv_w, tc_w, tc_b):
    x = np.ascontiguousarray(np.asarray(x, dtype=np.float32))
    conv_w = np.asarray(conv_w, dtype=np.float32)
    tc_w = np.asarray(tc_w, dtype=np.float32)
    tc_b = np.asarray(tc_b, dtype=np.float32)
    assert x.shape == (B, S, C), x.shape

    from concourse.bass_utils import run_bass_kernel_spmd

    if "nc" not in _BUILD_CACHE:
        _BUILD_CACHE["nc"] = build_bass()
    nc = _BUILD_CACHE["nc"]

    shared = make_shared_inputs(conv_w, tc_w, tc_b)
    in_maps = []
    for c in range(NCORES):
        m = dict(shared)
        m["xin"] = np.ascontiguousarray(x[c * BLOC : (c + 1) * BLOC])
        in_maps.append(m)

    trace = bool(int(os.environ.get("KERNEL_TRACE", "0")))
    if trace:
        _install_ntff_hook()
    res = run_bass_kernel_spmd(
        nc, in_maps, core_ids=list(range(NCORES)), trace=trace, trace_cores=[0]
    )
    if trace and res.exec_time_ns is not None:
        print(
            f"[kernel] HW exec time: {res.exec_time_ns} ns "
            f"(mean {res.mean_exec_time_ns} ns)"
        )
        kernel.last_exec_time_ns = res.exec_time_ns
        kernel.last_trace = res.instructions_and_trace
    out = np.concatenate([r["out"] for r in res.results], axis=0)
    return out


if __name__ == "__main__":
    build_bass()
    print("build ok")

